# revision 7
# baseline (speedup 1.0000x reference)
"""Bass/Trainium2 kernel for nn_FC_Classifier (box-pooled FC classifier).

Math: pred[n,k] = (1/area_n) * sum_{(h,w) in box_n} (fc_w @ feature_map)[k,h,w] + fc_b[k]

Strategy (8 cores, one chip):
  * Contract channels FIRST (2048 -> 150) with a matmul: G[w,h,k] — sharded
    over image rows h (24 rows/core).  This is the only phase that touches the
    302 MB feature map, so it is HBM-bound and fully parallel.
  * W-cumsum via triangular matmul (PE): Q[x,h,k], still h-sharded.
  * AllToAll: trade h-shards for x-shards (compact 2.9 MB buffers).
  * H-cumsum via triangular matmul: T[y, x_shard, k] = integral image of G.
  * AllGather the x-shards into the full integral image T[x,y,k] (shared buf).
  * 4-corner indirect-DMA gathers at host-precomputed box corners, combine on
    DVE, scale by 1/area; bias is added on the host.

Self-contained: only numpy + the concourse (Bass) runtime are imported.
"""

import os
import numpy as np

DS = 8.0
NCORES = 8

LAST_RESULTS = None  # BassKernelResults of the most recent run (for test.py)

_NC_CACHE = {}


def _chunks(total, size):
    return [(o, min(size, total - o)) for o in range(0, total, size)]


def _box_indices_np(anchors, scale, H, W):
    # exact replica of reference._box_indices in numpy f32
    a = anchors.astype(np.float32) / np.float32(DS)
    x0 = (a[:, 0] * scale[1]).astype(np.int32)
    x1 = (a[:, 1] * scale[1]).astype(np.int32)
    y0 = (a[:, 2] * scale[0]).astype(np.int32)
    y1 = (a[:, 3] * scale[0]).astype(np.int32)
    eqy = y0 == y1
    y0, y1 = (
        np.where(eqy & (y0 != 0), y0 - 1, y0),
        np.where(eqy & (y0 == 0), y1 + 1, y1),
    )
    eqx = x0 == x1
    x0, x1 = (
        np.where(eqx & (x0 != 0), x0 - 1, x0),
        np.where(eqx & (x0 == 0), x1 + 1, x1),
    )
    y0, y1 = np.clip(y0, 0, H), np.clip(y1, 0, H)
    x0, x1 = np.clip(x0, 0, W), np.clip(x1, 0, W)
    return x0, x1, y0, y1


def _build_nc(C, H, W, K, ACH, gather_batch):
    """Build + compile the SPMD Bass program (identical on all 8 cores).

    ACH = anchor chunks of 128 per core.  gather_batch = corners gathered per
    indirect DMA instruction (1 = one offset column per instruction).
    """
    from concourse import bacc, mybir, tile
    import concourse.bass as bass

    f32 = mybir.dt.float32
    i32 = mybir.dt.int32

    HSH = H // NCORES              # h rows per core
    YD = H + 1                     # y index range of integral image
    XP = -(-(W + 1) // NCORES) * NCORES  # x range padded to multiple of 8
    XSH = XP // NCORES             # x cols per core after AllToAll
    CCH = C // 128                 # channel chunks
    assert C % 128 == 0 and H % NCORES == 0

    wch = _chunks(W, 128)          # w partition chunks for G
    xch = _chunks(XP, 128)         # x partition chunks for Q
    ych = _chunks(YD, 128)         # y partition chunks for T
    # h-groups for the H-cumsum contraction: whole source blocks (HSH rows
    # each) packed into <=128 partitions
    bpg = max(1, 128 // HSH)       # blocks per group
    hgrp = _chunks(NCORES, bpg)    # (block_off, nblocks)

    NF2 = HSH * K                  # free size of G/Q tiles
    NF3 = XSH * K                  # free size of R/T tiles

    nc = bacc.Bacc("TRN2", target_bir_lowering=False, debug=False,
                   num_devices=NCORES)
    fm = nc.dram_tensor("fm", [C, HSH, W], f32, kind="ExternalInput").ap()
    fcw = nc.dram_tensor("fcw", [C, K], f32, kind="ExternalInput").ap()
    tri = nc.dram_tensor("tri", [W, XP], f32, kind="ExternalInput").ap()
    cidx = nc.dram_tensor("cidx", [4, 128, ACH], i32, kind="ExternalInput").ap()
    iar = nc.dram_tensor("iar", [128, ACH], f32, kind="ExternalInput").ap()
    pred = nc.dram_tensor("pred", [128 * ACH, K], f32, kind="ExternalOutput").ap()

    RG = [list(range(NCORES))]

    with tile.TileContext(nc) as tc:
        with (
            tc.tile_pool(name="constp", bufs=1) as constp,
            tc.tile_pool(name="fmp", bufs=3) as fmp,
            tc.tile_pool(name="bigp", bufs=1) as bigp,
            tc.tile_pool(name="psp", bufs=4, space="PSUM") as psp,
            tc.tile_pool(name="gatp", bufs=2) as gatp,
            tc.tile_pool(name="dramp", bufs=1, space="DRAM") as dramp,
        ):
            # ---- constants -------------------------------------------------
            fcw_sb = constp.tile([128, CCH * K], f32, tag="fcw", name="fcw_sb")
            nc.sync.dma_start(fcw_sb[:], fcw.rearrange("(cc p) k -> p cc k", p=128))

            tri_w = []                       # [wsz, XP] per w-chunk (phase 2 lhs-K rows)
            for j, (off, sz) in enumerate(wch):
                t = constp.tile([sz, XP], f32, tag=f"tri_w{j}", name=f"tri_w{j}")
                nc.sync.dma_start(t[:], tri[off:off + sz, :])
                tri_w.append(t)
            tri_h = []                       # [grows, YD] per h-group (phase 4)
            for j, (boff, nb) in enumerate(hgrp):
                r0, rn = boff * HSH, nb * HSH
                t = constp.tile([rn, YD], f32, tag=f"tri_h{j}", name=f"tri_h{j}")
                nc.sync.dma_start(t[:], tri[r0:r0 + rn, 0:YD])
                tri_h.append(t)

            idx_sb = constp.tile([128, 4 * ACH], i32, tag="idx", name="idx_sb")
            nc.sync.dma_start(idx_sb[:], cidx.rearrange("c p m -> p c m"))
            iar_sb = constp.tile([128, ACH], f32, tag="iar", name="iar_sb")
            nc.sync.dma_start(iar_sb[:], iar[:, :])

            # ---- phase 1: channel contraction  G[w, (h,k)] -----------------
            G = [bigp.tile([sz, NF2], f32, tag=f"G{j}", name=f"G{j}")
                 for j, (off, sz) in enumerate(wch)]
            fmv = fm.rearrange("(cc p) h w -> p cc h w", p=128)
            for h in range(HSH):
                fmh = fmp.tile([128, CCH * W], f32, tag="fmh", name="fmh")
                nc.sync.dma_start(fmh[:], fmv[:, :, h, :])
                for j, (woff, wsz) in enumerate(wch):
                    ps = psp.tile([wsz, K], f32, tag="ps", name="ps1")
                    for cc in range(CCH):
                        nc.tensor.matmul(
                            ps[:],
                            lhsT=fmh[:, cc * W + woff: cc * W + woff + wsz],
                            rhs=fcw_sb[:, cc * K:(cc + 1) * K],
                            start=(cc == 0), stop=(cc == CCH - 1),
                        )
                    nc.vector.tensor_copy(G[j][:, h * K:(h + 1) * K], ps[:])

            # ---- phase 2: W-cumsum  Q[x, (h,k)] ----------------------------
            Q = [bigp.tile([sz, NF2], f32, tag=f"Q{j}", name=f"Q{j}")
                 for j, (off, sz) in enumerate(xch)]
            for j, (xoff, xsz) in enumerate(xch):
                for n0, nsz in _chunks(NF2, 512):
                    ps = psp.tile([xsz, nsz], f32, tag="ps", name="ps2")
                    for wj, (woff, wsz) in enumerate(wch):
                        nc.tensor.matmul(
                            ps[:],
                            lhsT=tri_w[wj][:, xoff:xoff + xsz],
                            rhs=G[wj][:, n0:n0 + nsz],
                            start=(wj == 0), stop=(wj == len(wch) - 1),
                        )
                    nc.vector.tensor_copy(Q[j][:, n0:n0 + nsz], ps[:])

            # ---- AllToAll: h-shards -> x-shards ----------------------------
            a2a_in = dramp.tile([XP, NF2], f32, tag="a2a_in", name="a2a_in")
            a2a_out = dramp.tile([NCORES, XSH, HSH, K], f32, tag="a2a_out",
                                 name="a2a_out")
            for j, (xoff, xsz) in enumerate(xch):
                nc.sync.dma_start(a2a_in[xoff:xoff + xsz, :], Q[j][:])
            nc.gpsimd.collective_compute(
                "AllToAll", mybir.AluOpType.bypass, replica_groups=RG,
                ins=[a2a_in.opt()], outs=[a2a_out.opt()],
            )

            # ---- phase 3: H-cumsum  T[y, (x,k)] ----------------------------
            # R/T tiles reuse the G/Q slots (G and Q are dead by now)
            aov = a2a_out.rearrange("i x h k -> i h x k")
            R = []
            for j, (boff, nb) in enumerate(hgrp):
                t = bigp.tile([nb * HSH, NF3], f32, tag=f"G{j % len(wch)}",
                              name=f"R{j}")
                for b in range(nb):
                    nc.sync.dma_start(t[b * HSH:(b + 1) * HSH, :],
                                      aov[boff + b])
                R.append(t)
            T = [bigp.tile([sz, NF3], f32, tag=f"Q{j % len(xch)}", name=f"T{j}")
                 for j, (off, sz) in enumerate(ych)]
            for j, (yoff, ysz) in enumerate(ych):
                for n0, nsz in _chunks(NF3, 512):
                    ps = psp.tile([ysz, nsz], f32, tag="ps", name="ps3")
                    for gj in range(len(hgrp)):
                        nc.tensor.matmul(
                            ps[:],
                            lhsT=tri_h[gj][:, yoff:yoff + ysz],
                            rhs=R[gj][:, n0:n0 + nsz],
                            start=(gj == 0), stop=(gj == len(hgrp) - 1),
                        )
                    nc.vector.tensor_copy(T[j][:, n0:n0 + nsz], ps[:])

            # ---- AllGather full integral image T[x, y, k] ------------------
            ag_in = dramp.tile([XSH, YD, K], f32, tag="ag_in", name="ag_in")
            ag_out = dramp.tile([XP * YD, K], f32, tag="ag_out", name="ag_out",
                                addr_space="Shared")
            agv = ag_in.rearrange("x y k -> y x k")
            for j, (yoff, ysz) in enumerate(ych):
                nc.sync.dma_start(agv[yoff:yoff + ysz], T[j][:])
            nc.gpsimd.collective_compute(
                "AllGather", mybir.AluOpType.bypass, replica_groups=RG,
                ins=[ag_in.opt()], outs=[ag_out.opt()],
            )

            # ---- phase 4: corner gathers + combine -------------------------
            GB = gather_batch
            assert ACH % GB == 0
            pv = pred.rearrange("(m p) k -> p m k", p=128)
            for m0 in range(0, ACH, GB):
                g = []
                for c in range(4):
                    gt = gatp.tile([128, GB * K], f32, tag=f"g{c}", name=f"g{c}")
                    nc.gpsimd.indirect_dma_start(
                        out=gt[:],
                        out_offset=None,
                        in_=ag_out[:],
                        in_offset=bass.IndirectOffsetOnAxis(
                            ap=idx_sb[:, c * ACH + m0: c * ACH + m0 + GB],
                            axis=0,
                        ),
                    )
                    g.append(gt)
                # sums = g0 - g1 - g2 + g3, scaled by 1/area (in-place combine)
                nc.vector.tensor_sub(g[0][:], g[0][:], g[1][:])
                nc.vector.tensor_sub(g[2][:], g[2][:], g[3][:])
                nc.vector.tensor_sub(g[0][:], g[0][:], g[2][:])
                for b in range(GB):
                    nc.vector.tensor_scalar_mul(
                        g[1][:, b * K:(b + 1) * K], g[0][:, b * K:(b + 1) * K],
                        iar_sb[:, m0 + b: m0 + b + 1],
                    )
                # pred rows: anchor a = m*128 + p
                nc.sync.dma_start(pv[:, m0:m0 + GB, :], g[1][:])

    nc.compile()
    return nc


def _get_nc(C, H, W, K, ACH, gather_batch):
    key = (C, H, W, K, ACH, gather_batch)
    if key not in _NC_CACHE:
        _NC_CACHE[key] = _build_nc(C, H, W, K, ACH, gather_batch)
    return _NC_CACHE[key]


def _prepare(feature_map, scale, anchors, fc_w, anchor_num):
    """Host-side prep: shard fm, build tri matrix, corner indices, areas."""
    C, H, W = feature_map.shape
    K = fc_w.shape[0]
    HSH = H // NCORES
    YD = H + 1
    XP = -(-(W + 1) // NCORES) * NCORES

    N = int(anchor_num)
    anchors = np.asarray(anchors, dtype=np.float32)[:N]
    per = 128 * NCORES
    Npad = max(per, -(-N // per) * per)
    if Npad != N:
        pad = np.zeros((Npad - N, 4), dtype=np.float32)
        anchors = np.concatenate([anchors, pad], axis=0)
    ACH = Npad // per

    x0, x1, y0, y1 = _box_indices_np(anchors, np.asarray(scale, np.float32), H, W)
    area = np.maximum((y1 - y0) * (x1 - x0), 1).astype(np.float32)
    inv_area = (np.float32(1.0) / area).astype(np.float32)
    # integral image is stored [x, y, k]; row id = x*YD + y
    corners = np.stack([
        x1 * YD + y1,
        x1 * YD + y0,
        x0 * YD + y1,
        x0 * YD + y0,
    ]).astype(np.int32)                      # [4, Npad]

    fcwT = np.ascontiguousarray(fc_w.T.astype(np.float32))      # [C, K]
    tri = np.zeros((W, XP), dtype=np.float32)
    for x in range(1, W + 1):
        tri[0:x, x] = 1.0

    ash = Npad // NCORES
    in_maps = []
    for i in range(NCORES):
        fm_i = np.ascontiguousarray(feature_map[:, i * HSH:(i + 1) * HSH, :])
        c_i = corners[:, i * ash:(i + 1) * ash]                  # [4, ash]
        c_i = c_i.reshape(4, ACH, 128).transpose(0, 2, 1)        # [4,128,ACH]
        a_i = inv_area[i * ash:(i + 1) * ash].reshape(ACH, 128).T  # [128,ACH]
        in_maps.append({
            "fm": fm_i,
            "fcw": fcwT,
            "tri": tri,
            "cidx": np.ascontiguousarray(c_i),
            "iar": np.ascontiguousarray(a_i),
        })
    return in_maps, ACH, N, Npad, K


def kernel(**inputs):
    global LAST_RESULTS
    feature_map = np.asarray(inputs["feature_map"], dtype=np.float32)
    scale = np.asarray(inputs["scale"], dtype=np.float32)
    anchors = np.asarray(inputs["anchors"], dtype=np.float32)
    fc_w = np.asarray(inputs["fc_w"], dtype=np.float32)
    fc_b = np.asarray(inputs["fc_b"], dtype=np.float32)
    anchor_num = int(np.asarray(inputs["anchor_num"]))

    C, H, W = feature_map.shape
    K = fc_w.shape[0]

    import time
    t0 = time.time()
    in_maps, ACH, N, Npad, K = _prepare(feature_map, scale, anchors, fc_w,
                                        anchor_num)
    print(f"[kernel] host prep {time.time() - t0:.1f}s", flush=True)
    gather_batch = int(os.environ.get("NMS_GATHER_BATCH", "1"))
    while ACH % gather_batch:
        gather_batch //= 2
    t0 = time.time()
    nc = _get_nc(C, H, W, K, ACH, gather_batch)
    print(f"[kernel] bass build+schedule {time.time() - t0:.1f}s", flush=True)

    from concourse.bass_utils import run_bass_kernel_spmd
    trace = bool(int(os.environ.get("NMS_TRACE", "0")))
    t0 = time.time()
    res = run_bass_kernel_spmd(nc, in_maps, core_ids=list(range(NCORES)),
                               trace=trace)
    print(f"[kernel] compile+run {time.time() - t0:.1f}s", flush=True)
    LAST_RESULTS = res
    pred = np.concatenate([res.results[i]["pred"] for i in range(NCORES)],
                          axis=0)[:N]
    return (pred + fc_b[None, :].astype(np.float32)).astype(np.float32)


# revision 12
# speedup vs baseline: 1.3411x; 1.3411x over previous
"""Bass/Trainium2 kernel for nn_FC_Classifier (box-pooled FC classifier).

Math: pred[n,k] = (1/area_n) * sum_{(h,w) in box_n} (fc_w @ feature_map)[k,h,w] + fc_b[k]

Strategy (8 cores, one chip):
  * Contract channels FIRST (2048 -> 150) with a matmul: G[w,h,k] — sharded
    over image rows h (24 rows/core).  This is the only phase that touches the
    302 MB feature map, so it is HBM-bound and fully parallel.
  * W-cumsum via triangular matmul (PE): Q[x,h,k], still h-sharded.
  * AllToAll: trade h-shards for x-shards (compact 2.9 MB buffers).
  * H-cumsum via triangular matmul: T[y, x_shard, k] = integral image of G.
  * AllGather the x-shards into the full integral image T[x,y,k] (shared buf).
  * 4-corner indirect-DMA gathers at host-precomputed box corners, combine on
    DVE, scale by 1/area; bias is added on the host.

Self-contained: only numpy + the concourse (Bass) runtime are imported.
"""

import os
import numpy as np

DS = 8.0
NCORES = 8

LAST_RESULTS = None  # BassKernelResults of the most recent run (for test.py)

_NC_CACHE = {}


def _chunks(total, size):
    return [(o, min(size, total - o)) for o in range(0, total, size)]


def _box_indices_np(anchors, scale, H, W):
    # exact replica of reference._box_indices in numpy f32
    a = anchors.astype(np.float32) / np.float32(DS)
    x0 = (a[:, 0] * scale[1]).astype(np.int32)
    x1 = (a[:, 1] * scale[1]).astype(np.int32)
    y0 = (a[:, 2] * scale[0]).astype(np.int32)
    y1 = (a[:, 3] * scale[0]).astype(np.int32)
    eqy = y0 == y1
    y0, y1 = (
        np.where(eqy & (y0 != 0), y0 - 1, y0),
        np.where(eqy & (y0 == 0), y1 + 1, y1),
    )
    eqx = x0 == x1
    x0, x1 = (
        np.where(eqx & (x0 != 0), x0 - 1, x0),
        np.where(eqx & (x0 == 0), x1 + 1, x1),
    )
    y0, y1 = np.clip(y0, 0, H), np.clip(y1, 0, H)
    x0, x1 = np.clip(x0, 0, W), np.clip(x1, 0, W)
    return x0, x1, y0, y1


def _build_nc(C, H, W, K, ACH, gather_batch, bf16):
    """Build + compile the SPMD Bass program (identical on all 8 cores).

    ACH = anchor chunks of 128 per core.  gather_batch = corners gathered per
    indirect DMA instruction (1 = one offset column per instruction).
    bf16: feature_map/fc_w arrive as bf16 (1-pass matmuls + FWL, half the
    HBM traffic); the integral-image math stays f32.
    """
    from concourse import bacc, mybir, tile
    import concourse.bass as bass

    f32 = mybir.dt.float32
    fmdt = mybir.dt.bfloat16 if bf16 else f32
    i32 = mybir.dt.int32

    HSH = H // NCORES              # h rows per core
    YD = H + 1                     # y index range of integral image
    XP = -(-(W + 1) // NCORES) * NCORES  # x range padded to multiple of 8
    XSH = XP // NCORES             # x cols per core after AllToAll
    CCH = C // 128                 # channel chunks
    assert C % 128 == 0 and H % NCORES == 0

    wch = _chunks(W, 128)          # w partition chunks for G
    xch = _chunks(XP, 128)         # x partition chunks for Q
    ych = _chunks(YD, 128)         # y partition chunks for T
    # h-groups for the H-cumsum contraction: whole source blocks (HSH rows
    # each) packed into <=128 partitions
    bpg = max(1, 128 // HSH)       # blocks per group
    hgrp = _chunks(NCORES, bpg)    # (block_off, nblocks)

    NF2 = HSH * K                  # free size of G/Q tiles
    NF3 = XSH * K                  # free size of R/T tiles

    nc = bacc.Bacc("TRN2", target_bir_lowering=False, debug=False,
                   num_devices=NCORES)
    fm = nc.dram_tensor("fm", [C, HSH, W], fmdt, kind="ExternalInput").ap()
    fcw = nc.dram_tensor("fcw", [C, K], fmdt, kind="ExternalInput").ap()
    tri = nc.dram_tensor("tri", [W, XP], f32, kind="ExternalInput").ap()
    cidx = nc.dram_tensor("cidx", [4, 128, ACH], i32, kind="ExternalInput").ap()
    iar = nc.dram_tensor("iar", [128, ACH], f32, kind="ExternalInput").ap()
    pred = nc.dram_tensor("pred", [128 * ACH, K], f32, kind="ExternalOutput").ap()

    RG = [list(range(NCORES))]

    with tile.TileContext(nc) as tc:
        with (
            tc.tile_pool(name="constp", bufs=1) as constp,
            tc.tile_pool(name="fmp", bufs=3) as fmp,
            tc.tile_pool(name="bigp", bufs=1) as bigp,
            tc.tile_pool(name="psp", bufs=4, space="PSUM") as psp,
            tc.tile_pool(name="gatp", bufs=2) as gatp,
            tc.tile_pool(name="dramp", bufs=1, space="DRAM") as dramp,
        ):
            # ---- constants -------------------------------------------------
            fcw_sb = constp.tile([128, CCH * K], fmdt, tag="fcw", name="fcw_sb")
            nc.sync.dma_start(fcw_sb[:], fcw.rearrange("(cc p) k -> p cc k", p=128))

            tri_w = []                       # [wsz, XP] per w-chunk (phase 2 lhs-K rows)
            for j, (off, sz) in enumerate(wch):
                t = constp.tile([sz, XP], f32, tag=f"tri_w{j}", name=f"tri_w{j}")
                nc.sync.dma_start(t[:], tri[off:off + sz, :])
                tri_w.append(t)
            tri_h = []                       # [grows, YD] per h-group (phase 4)
            for j, (boff, nb) in enumerate(hgrp):
                r0, rn = boff * HSH, nb * HSH
                t = constp.tile([rn, YD], f32, tag=f"tri_h{j}", name=f"tri_h{j}")
                nc.sync.dma_start(t[:], tri[r0:r0 + rn, 0:YD])
                tri_h.append(t)

            idx_sb = constp.tile([128, 4 * ACH], i32, tag="idx", name="idx_sb")
            nc.sync.dma_start(idx_sb[:], cidx.rearrange("c p m -> p c m"))
            iar_sb = constp.tile([128, ACH], f32, tag="iar", name="iar_sb")
            nc.sync.dma_start(iar_sb[:], iar[:, :])

            # ---- phase 1: channel contraction  G[w, (h,k)] -----------------
            G = [bigp.tile([sz, NF2], f32, tag=f"G{j}", name=f"G{j}")
                 for j, (off, sz) in enumerate(wch)]
            # load HB image rows per DMA so per-partition runs stay >=512B
            HB = 2 if HSH % 2 == 0 else 1
            fmv = fm.rearrange("(cc p) h w -> p cc (h w)", p=128)
            for h0 in range(0, HSH, HB):
                fmh = fmp.tile([128, CCH * HB * W], fmdt, tag="fmh", name="fmh")
                nc.sync.dma_start(fmh[:], fmv[:, :, h0 * W:(h0 + HB) * W])
                for hh in range(HB):
                    h = h0 + hh
                    for j, (woff, wsz) in enumerate(wch):
                        ps = psp.tile([wsz, K], f32, tag="ps", name="ps1")
                        for cc in range(CCH):
                            o = cc * (HB * W) + hh * W + woff
                            nc.tensor.matmul(
                                ps[:],
                                lhsT=fmh[:, o: o + wsz],
                                rhs=fcw_sb[:, cc * K:(cc + 1) * K],
                                start=(cc == 0), stop=(cc == CCH - 1),
                            )
                        nc.vector.tensor_copy(G[j][:, h * K:(h + 1) * K], ps[:])

            # ---- phase 2: W-cumsum  Q[x, (h,k)] ----------------------------
            Q = [bigp.tile([sz, NF2], f32, tag=f"Q{j}", name=f"Q{j}")
                 for j, (off, sz) in enumerate(xch)]
            for j, (xoff, xsz) in enumerate(xch):
                for n0, nsz in _chunks(NF2, 512):
                    ps = psp.tile([xsz, nsz], f32, tag="ps", name="ps2")
                    for wj, (woff, wsz) in enumerate(wch):
                        nc.tensor.matmul(
                            ps[:],
                            lhsT=tri_w[wj][:, xoff:xoff + xsz],
                            rhs=G[wj][:, n0:n0 + nsz],
                            start=(wj == 0), stop=(wj == len(wch) - 1),
                        )
                    nc.vector.tensor_copy(Q[j][:, n0:n0 + nsz], ps[:])

            # ---- AllToAll: h-shards -> x-shards ----------------------------
            a2a_in = dramp.tile([XP, NF2], f32, tag="a2a_in", name="a2a_in")
            a2a_out = dramp.tile([NCORES, XSH, HSH, K], f32, tag="a2a_out",
                                 name="a2a_out")
            for j, (xoff, xsz) in enumerate(xch):
                nc.sync.dma_start(a2a_in[xoff:xoff + xsz, :], Q[j][:])
            nc.gpsimd.collective_compute(
                "AllToAll", mybir.AluOpType.bypass, replica_groups=RG,
                ins=[a2a_in.opt()], outs=[a2a_out.opt()],
            )

            # ---- phase 3: H-cumsum  T[y, (x,k)] ----------------------------
            # R/T tiles reuse the G/Q slots (G and Q are dead by now)
            aov = a2a_out.rearrange("i x h k -> i h x k")
            R = []
            for j, (boff, nb) in enumerate(hgrp):
                t = bigp.tile([nb * HSH, NF3], f32, tag=f"G{j % len(wch)}",
                              name=f"R{j}")
                for b in range(nb):
                    nc.sync.dma_start(t[b * HSH:(b + 1) * HSH, :],
                                      aov[boff + b])
                R.append(t)
            T = [bigp.tile([sz, NF3], f32, tag=f"Q{j % len(xch)}", name=f"T{j}")
                 for j, (off, sz) in enumerate(ych)]
            for j, (yoff, ysz) in enumerate(ych):
                for n0, nsz in _chunks(NF3, 512):
                    ps = psp.tile([ysz, nsz], f32, tag="ps", name="ps3")
                    for gj in range(len(hgrp)):
                        nc.tensor.matmul(
                            ps[:],
                            lhsT=tri_h[gj][:, yoff:yoff + ysz],
                            rhs=R[gj][:, n0:n0 + nsz],
                            start=(gj == 0), stop=(gj == len(hgrp) - 1),
                        )
                    nc.vector.tensor_copy(T[j][:, n0:n0 + nsz], ps[:])

            # ---- AllGather full integral image T[x, y, k] ------------------
            ag_in = dramp.tile([XSH, YD, K], f32, tag="ag_in", name="ag_in")
            ag_out = dramp.tile([XP * YD, K], f32, tag="ag_out", name="ag_out",
                                addr_space="Shared")
            agv = ag_in.rearrange("x y k -> y x k")
            for j, (yoff, ysz) in enumerate(ych):
                nc.sync.dma_start(agv[yoff:yoff + ysz], T[j][:])
            nc.gpsimd.collective_compute(
                "AllGather", mybir.AluOpType.bypass, replica_groups=RG,
                ins=[ag_in.opt()], outs=[ag_out.opt()],
            )

            # ---- phase 4: corner gathers + combine -------------------------
            GB = gather_batch
            assert ACH % GB == 0
            pv = pred.rearrange("(m p) k -> p m k", p=128)
            for m0 in range(0, ACH, GB):
                g = []
                for c in range(4):
                    gt = gatp.tile([128, GB * K], f32, tag=f"g{c}", name=f"g{c}")
                    nc.gpsimd.indirect_dma_start(
                        out=gt[:],
                        out_offset=None,
                        in_=ag_out[:],
                        in_offset=bass.IndirectOffsetOnAxis(
                            ap=idx_sb[:, c * ACH + m0: c * ACH + m0 + GB],
                            axis=0,
                        ),
                    )
                    g.append(gt)
                # sums = g0 - g1 - g2 + g3, scaled by 1/area (in-place combine)
                nc.vector.tensor_sub(g[0][:], g[0][:], g[1][:])
                nc.vector.tensor_sub(g[2][:], g[2][:], g[3][:])
                nc.vector.tensor_sub(g[0][:], g[0][:], g[2][:])
                for b in range(GB):
                    nc.vector.tensor_scalar_mul(
                        g[1][:, b * K:(b + 1) * K], g[0][:, b * K:(b + 1) * K],
                        iar_sb[:, m0 + b: m0 + b + 1],
                    )
                # pred rows: anchor a = m*128 + p
                nc.sync.dma_start(pv[:, m0:m0 + GB, :], g[1][:])

    nc.compile()
    return nc


def _get_nc(C, H, W, K, ACH, gather_batch, bf16):
    key = (C, H, W, K, ACH, gather_batch, bf16)
    if key not in _NC_CACHE:
        _NC_CACHE[key] = _build_nc(C, H, W, K, ACH, gather_batch, bf16)
    return _NC_CACHE[key]


def _prepare(feature_map, scale, anchors, fc_w, anchor_num, bf16):
    """Host-side prep: shard fm, build tri matrix, corner indices, areas."""
    C, H, W = feature_map.shape
    K = fc_w.shape[0]
    HSH = H // NCORES
    YD = H + 1
    XP = -(-(W + 1) // NCORES) * NCORES

    N = int(anchor_num)
    anchors = np.asarray(anchors, dtype=np.float32)[:N]
    per = 128 * NCORES
    Npad = max(per, -(-N // per) * per)
    if Npad != N:
        pad = np.zeros((Npad - N, 4), dtype=np.float32)
        anchors = np.concatenate([anchors, pad], axis=0)
    ACH = Npad // per

    x0, x1, y0, y1 = _box_indices_np(anchors, np.asarray(scale, np.float32), H, W)
    area = np.maximum((y1 - y0) * (x1 - x0), 1).astype(np.float32)
    inv_area = (np.float32(1.0) / area).astype(np.float32)
    # integral image is stored [x, y, k]; row id = x*YD + y
    corners = np.stack([
        x1 * YD + y1,
        x1 * YD + y0,
        x0 * YD + y1,
        x0 * YD + y0,
    ]).astype(np.int32)                      # [4, Npad]

    if bf16:
        import ml_dtypes
        fdt = ml_dtypes.bfloat16
    else:
        fdt = np.float32
    fcwT = np.ascontiguousarray(fc_w.T.astype(fdt))              # [C, K]
    tri = np.zeros((W, XP), dtype=np.float32)
    for x in range(1, W + 1):
        tri[0:x, x] = 1.0

    ash = Npad // NCORES
    in_maps = []
    for i in range(NCORES):
        fm_i = np.ascontiguousarray(feature_map[:, i * HSH:(i + 1) * HSH, :].astype(fdt))
        c_i = corners[:, i * ash:(i + 1) * ash]                  # [4, ash]
        c_i = c_i.reshape(4, ACH, 128).transpose(0, 2, 1)        # [4,128,ACH]
        a_i = inv_area[i * ash:(i + 1) * ash].reshape(ACH, 128).T  # [128,ACH]
        in_maps.append({
            "fm": fm_i,
            "fcw": fcwT,
            "tri": tri,
            "cidx": np.ascontiguousarray(c_i),
            "iar": np.ascontiguousarray(a_i),
        })
    return in_maps, ACH, N, Npad, K


def kernel(**inputs):
    global LAST_RESULTS
    feature_map = np.asarray(inputs["feature_map"], dtype=np.float32)
    scale = np.asarray(inputs["scale"], dtype=np.float32)
    anchors = np.asarray(inputs["anchors"], dtype=np.float32)
    fc_w = np.asarray(inputs["fc_w"], dtype=np.float32)
    fc_b = np.asarray(inputs["fc_b"], dtype=np.float32)
    anchor_num = int(np.asarray(inputs["anchor_num"]))

    C, H, W = feature_map.shape
    K = fc_w.shape[0]

    import time
    t0 = time.time()
    bf16 = bool(int(os.environ.get("NMS_BF16", "1")))
    in_maps, ACH, N, Npad, K = _prepare(feature_map, scale, anchors, fc_w,
                                        anchor_num, bf16)
    print(f"[kernel] host prep {time.time() - t0:.1f}s", flush=True)
    gather_batch = int(os.environ.get("NMS_GATHER_BATCH", "1"))
    while ACH % gather_batch:
        gather_batch //= 2
    t0 = time.time()
    nc = _get_nc(C, H, W, K, ACH, gather_batch, bf16)
    print(f"[kernel] bass build+schedule {time.time() - t0:.1f}s", flush=True)

    from concourse.bass_utils import run_bass_kernel_spmd
    trace = bool(int(os.environ.get("NMS_TRACE", "0")))
    t0 = time.time()
    res = run_bass_kernel_spmd(nc, in_maps, core_ids=list(range(NCORES)),
                               trace=trace)
    print(f"[kernel] compile+run {time.time() - t0:.1f}s", flush=True)
    LAST_RESULTS = res
    pred = np.concatenate([res.results[i]["pred"] for i in range(NCORES)],
                          axis=0)[:N]
    return (pred + fc_b[None, :].astype(np.float32)).astype(np.float32)


# revision 13
# speedup vs baseline: 1.4241x; 1.0618x over previous
"""Bass/Trainium2 kernel for nn_FC_Classifier (box-pooled FC classifier).

Math: pred[n,k] = (1/area_n) * sum_{(h,w) in box_n} (fc_w @ feature_map)[k,h,w] + fc_b[k]

Strategy (8 cores, one chip):
  * Contract channels FIRST (2048 -> 150) with a matmul: G[w,h,k] — sharded
    over image rows h (24 rows/core).  This is the only phase that touches the
    302 MB feature map, so it is HBM-bound and fully parallel.
  * W-cumsum via triangular matmul (PE): Q[x,h,k], still h-sharded.
  * AllToAll: trade h-shards for x-shards (compact 2.9 MB buffers).
  * H-cumsum via triangular matmul: T[y, x_shard, k] = integral image of G.
  * AllGather the x-shards into the full integral image T[x,y,k] (shared buf).
  * 4-corner indirect-DMA gathers at host-precomputed box corners, combine on
    DVE, scale by 1/area; bias is added on the host.

Self-contained: only numpy + the concourse (Bass) runtime are imported.
"""

import os
import numpy as np

DS = 8.0
NCORES = 8

LAST_RESULTS = None  # BassKernelResults of the most recent run (for test.py)

_NC_CACHE = {}


def _chunks(total, size):
    return [(o, min(size, total - o)) for o in range(0, total, size)]


def _box_indices_np(anchors, scale, H, W):
    # exact replica of reference._box_indices in numpy f32
    a = anchors.astype(np.float32) / np.float32(DS)
    x0 = (a[:, 0] * scale[1]).astype(np.int32)
    x1 = (a[:, 1] * scale[1]).astype(np.int32)
    y0 = (a[:, 2] * scale[0]).astype(np.int32)
    y1 = (a[:, 3] * scale[0]).astype(np.int32)
    eqy = y0 == y1
    y0, y1 = (
        np.where(eqy & (y0 != 0), y0 - 1, y0),
        np.where(eqy & (y0 == 0), y1 + 1, y1),
    )
    eqx = x0 == x1
    x0, x1 = (
        np.where(eqx & (x0 != 0), x0 - 1, x0),
        np.where(eqx & (x0 == 0), x1 + 1, x1),
    )
    y0, y1 = np.clip(y0, 0, H), np.clip(y1, 0, H)
    x0, x1 = np.clip(x0, 0, W), np.clip(x1, 0, W)
    return x0, x1, y0, y1


def _build_nc(C, H, W, K, ACH, gather_batch, bf16):
    """Build + compile the SPMD Bass program (identical on all 8 cores).

    ACH = anchor chunks of 128 per core.  gather_batch = corners gathered per
    indirect DMA instruction (1 = one offset column per instruction).
    bf16: feature_map/fc_w arrive as bf16 (1-pass matmuls + FWL, half the
    HBM traffic); the integral-image math stays f32.
    """
    from concourse import bacc, mybir, tile
    import concourse.bass as bass

    f32 = mybir.dt.float32
    fmdt = mybir.dt.bfloat16 if bf16 else f32
    i32 = mybir.dt.int32

    HSH = H // NCORES              # h rows per core
    YD = H + 1                     # y index range of integral image
    XP = -(-(W + 1) // NCORES) * NCORES  # x range padded to multiple of 8
    XSH = XP // NCORES             # x cols per core after AllToAll
    CCH = C // 128                 # channel chunks
    assert C % 128 == 0 and H % NCORES == 0

    wch = _chunks(W, 128)          # w partition chunks for G
    xch = _chunks(XP, 128)         # x partition chunks for Q
    ych = _chunks(YD, 128)         # y partition chunks for T
    # h-groups for the H-cumsum contraction: whole source blocks (HSH rows
    # each) packed into <=128 partitions
    bpg = max(1, 128 // HSH)       # blocks per group
    hgrp = _chunks(NCORES, bpg)    # (block_off, nblocks)

    NF2 = HSH * K                  # free size of G/Q tiles
    NF3 = XSH * K                  # free size of R/T tiles

    nc = bacc.Bacc("TRN2", target_bir_lowering=False, debug=False,
                   num_devices=NCORES)
    fm = nc.dram_tensor("fm", [C, HSH, W], fmdt, kind="ExternalInput").ap()
    fcw = nc.dram_tensor("fcw", [C, K], fmdt, kind="ExternalInput").ap()
    tri = nc.dram_tensor("tri", [W, XP], f32, kind="ExternalInput").ap()
    trib = nc.dram_tensor("trib", [W, XP], fmdt, kind="ExternalInput").ap()
    cidx = nc.dram_tensor("cidx", [4, 128, ACH], i32, kind="ExternalInput").ap()
    iar = nc.dram_tensor("iar", [128, ACH], f32, kind="ExternalInput").ap()
    pred = nc.dram_tensor("pred", [128 * ACH, K], f32, kind="ExternalOutput").ap()

    RG = [list(range(NCORES))]

    with tile.TileContext(nc) as tc:
        with (
            tc.tile_pool(name="constp", bufs=1) as constp,
            tc.tile_pool(name="fmp", bufs=4) as fmp,
            tc.tile_pool(name="bigp", bufs=1) as bigp,
            tc.tile_pool(name="psp", bufs=4, space="PSUM") as psp,
            tc.tile_pool(name="gatp", bufs=2) as gatp,
            tc.tile_pool(name="dramp", bufs=1, space="DRAM") as dramp,
        ):
            # ---- constants -------------------------------------------------
            fcw_sb = constp.tile([128, CCH * K], fmdt, tag="fcw", name="fcw_sb")
            nc.sync.dma_start(fcw_sb[:], fcw.rearrange("(cc p) k -> p cc k", p=128))

            tri_w = []                       # [wsz, XP] per w-chunk (phase 2 lhs-K rows)
            for j, (off, sz) in enumerate(wch):
                t = constp.tile([sz, XP], fmdt, tag=f"tri_w{j}", name=f"tri_w{j}")
                nc.sync.dma_start(t[:], trib[off:off + sz, :])
                tri_w.append(t)
            tri_h = []                       # [grows, YD] per h-group (phase 4)
            for j, (boff, nb) in enumerate(hgrp):
                r0, rn = boff * HSH, nb * HSH
                t = constp.tile([rn, YD], f32, tag=f"tri_h{j}", name=f"tri_h{j}")
                nc.sync.dma_start(t[:], tri[r0:r0 + rn, 0:YD])
                tri_h.append(t)

            idx_sb = constp.tile([128, 4 * ACH], i32, tag="idx", name="idx_sb")
            nc.sync.dma_start(idx_sb[:], cidx.rearrange("c p m -> p c m"))
            iar_sb = constp.tile([128, ACH], f32, tag="iar", name="iar_sb")
            nc.sync.dma_start(iar_sb[:], iar[:, :])

            # ---- phase 1: channel contraction  G[w, (h,k)] -----------------
            G = [bigp.tile([sz, NF2], fmdt, tag=f"G{j}", name=f"G{j}")
                 for j, (off, sz) in enumerate(wch)]
            # load HB image rows per DMA so per-partition runs stay >=512B
            HB = 2 if HSH % 2 == 0 else 1
            fmv = fm.rearrange("(cc p) h w -> p cc (h w)", p=128)
            for h0 in range(0, HSH, HB):
                fmh = fmp.tile([128, CCH * HB * W], fmdt, tag="fmh", name="fmh")
                nc.sync.dma_start(fmh[:], fmv[:, :, h0 * W:(h0 + HB) * W])
                for hh in range(HB):
                    h = h0 + hh
                    for j, (woff, wsz) in enumerate(wch):
                        ps = psp.tile([wsz, K], f32, tag="ps", name="ps1")
                        for cc in range(CCH):
                            o = cc * (HB * W) + hh * W + woff
                            nc.tensor.matmul(
                                ps[:],
                                lhsT=fmh[:, o: o + wsz],
                                rhs=fcw_sb[:, cc * K:(cc + 1) * K],
                                start=(cc == 0), stop=(cc == CCH - 1),
                            )
                        nc.vector.tensor_copy(G[j][:, h * K:(h + 1) * K], ps[:])

            # ---- phase 2: W-cumsum  Q[x, (h,k)] ----------------------------
            Q = [bigp.tile([sz, NF2], f32, tag=f"Q{j}", name=f"Q{j}")
                 for j, (off, sz) in enumerate(xch)]
            for j, (xoff, xsz) in enumerate(xch):
                for n0, nsz in _chunks(NF2, 512):
                    ps = psp.tile([xsz, nsz], f32, tag="ps", name="ps2")
                    for wj, (woff, wsz) in enumerate(wch):
                        nc.tensor.matmul(
                            ps[:],
                            lhsT=tri_w[wj][:, xoff:xoff + xsz],
                            rhs=G[wj][:, n0:n0 + nsz],
                            start=(wj == 0), stop=(wj == len(wch) - 1),
                        )
                    nc.vector.tensor_copy(Q[j][:, n0:n0 + nsz], ps[:])

            # ---- AllToAll: h-shards -> x-shards ----------------------------
            a2a_in = dramp.tile([XP, NF2], f32, tag="a2a_in", name="a2a_in")
            a2a_out = dramp.tile([NCORES, XSH, HSH, K], f32, tag="a2a_out",
                                 name="a2a_out")
            for j, (xoff, xsz) in enumerate(xch):
                nc.sync.dma_start(a2a_in[xoff:xoff + xsz, :], Q[j][:])
            nc.gpsimd.collective_compute(
                "AllToAll", mybir.AluOpType.bypass, replica_groups=RG,
                ins=[a2a_in.opt()], outs=[a2a_out.opt()],
            )

            # ---- phase 3: H-cumsum  T[y, (x,k)] ----------------------------
            # R/T tiles reuse the G/Q slots (G and Q are dead by now)
            aov = a2a_out.rearrange("i x h k -> i h x k")
            R = []
            for j, (boff, nb) in enumerate(hgrp):
                t = bigp.tile([nb * HSH, NF3], f32, tag=f"G{j % len(wch)}",
                              name=f"R{j}")
                for b in range(nb):
                    nc.sync.dma_start(t[b * HSH:(b + 1) * HSH, :],
                                      aov[boff + b])
                R.append(t)
            T = [bigp.tile([sz, NF3], f32, tag=f"Q{j % len(xch)}", name=f"T{j}")
                 for j, (off, sz) in enumerate(ych)]
            for j, (yoff, ysz) in enumerate(ych):
                for n0, nsz in _chunks(NF3, 512):
                    ps = psp.tile([ysz, nsz], f32, tag="ps", name="ps3")
                    for gj in range(len(hgrp)):
                        nc.tensor.matmul(
                            ps[:],
                            lhsT=tri_h[gj][:, yoff:yoff + ysz],
                            rhs=R[gj][:, n0:n0 + nsz],
                            start=(gj == 0), stop=(gj == len(hgrp) - 1),
                        )
                    nc.vector.tensor_copy(T[j][:, n0:n0 + nsz], ps[:])

            # ---- AllGather full integral image T[x, y, k] ------------------
            ag_in = dramp.tile([XSH, YD, K], f32, tag="ag_in", name="ag_in")
            ag_out = dramp.tile([XP * YD, K], f32, tag="ag_out", name="ag_out",
                                addr_space="Shared")
            agv = ag_in.rearrange("x y k -> y x k")
            for j, (yoff, ysz) in enumerate(ych):
                nc.sync.dma_start(agv[yoff:yoff + ysz], T[j][:])
            nc.gpsimd.collective_compute(
                "AllGather", mybir.AluOpType.bypass, replica_groups=RG,
                ins=[ag_in.opt()], outs=[ag_out.opt()],
            )

            # ---- phase 4: corner gathers + combine -------------------------
            GB = gather_batch
            assert ACH % GB == 0
            pv = pred.rearrange("(m p) k -> p m k", p=128)
            for m0 in range(0, ACH, GB):
                g = []
                for c in range(4):
                    gt = gatp.tile([128, GB * K], f32, tag=f"g{c}", name=f"g{c}")
                    nc.gpsimd.indirect_dma_start(
                        out=gt[:],
                        out_offset=None,
                        in_=ag_out[:],
                        in_offset=bass.IndirectOffsetOnAxis(
                            ap=idx_sb[:, c * ACH + m0: c * ACH + m0 + GB],
                            axis=0,
                        ),
                    )
                    g.append(gt)
                # sums = g0 - g1 - g2 + g3, scaled by 1/area (in-place combine)
                nc.vector.tensor_sub(g[0][:], g[0][:], g[1][:])
                nc.vector.tensor_sub(g[2][:], g[2][:], g[3][:])
                nc.vector.tensor_sub(g[0][:], g[0][:], g[2][:])
                for b in range(GB):
                    nc.vector.tensor_scalar_mul(
                        g[1][:, b * K:(b + 1) * K], g[0][:, b * K:(b + 1) * K],
                        iar_sb[:, m0 + b: m0 + b + 1],
                    )
                # pred rows: anchor a = m*128 + p
                nc.sync.dma_start(pv[:, m0:m0 + GB, :], g[1][:])

    nc.compile()
    return nc


def _get_nc(C, H, W, K, ACH, gather_batch, bf16):
    key = (C, H, W, K, ACH, gather_batch, bf16)
    if key not in _NC_CACHE:
        _NC_CACHE[key] = _build_nc(C, H, W, K, ACH, gather_batch, bf16)
    return _NC_CACHE[key]


def _prepare(feature_map, scale, anchors, fc_w, anchor_num, bf16):
    """Host-side prep: shard fm, build tri matrix, corner indices, areas."""
    C, H, W = feature_map.shape
    K = fc_w.shape[0]
    HSH = H // NCORES
    YD = H + 1
    XP = -(-(W + 1) // NCORES) * NCORES

    N = int(anchor_num)
    anchors = np.asarray(anchors, dtype=np.float32)[:N]
    per = 128 * NCORES
    Npad = max(per, -(-N // per) * per)
    if Npad != N:
        pad = np.zeros((Npad - N, 4), dtype=np.float32)
        anchors = np.concatenate([anchors, pad], axis=0)
    ACH = Npad // per

    x0, x1, y0, y1 = _box_indices_np(anchors, np.asarray(scale, np.float32), H, W)
    area = np.maximum((y1 - y0) * (x1 - x0), 1).astype(np.float32)
    inv_area = (np.float32(1.0) / area).astype(np.float32)
    # integral image is stored [x, y, k]; row id = x*YD + y
    corners = np.stack([
        x1 * YD + y1,
        x1 * YD + y0,
        x0 * YD + y1,
        x0 * YD + y0,
    ]).astype(np.int32)                      # [4, Npad]

    if bf16:
        import ml_dtypes
        fdt = ml_dtypes.bfloat16
    else:
        fdt = np.float32
    fcwT = np.ascontiguousarray(fc_w.T.astype(fdt))              # [C, K]
    tri = np.zeros((W, XP), dtype=np.float32)
    for x in range(1, W + 1):
        tri[0:x, x] = 1.0

    ash = Npad // NCORES
    in_maps = []
    for i in range(NCORES):
        fm_i = np.ascontiguousarray(feature_map[:, i * HSH:(i + 1) * HSH, :].astype(fdt))
        c_i = corners[:, i * ash:(i + 1) * ash]                  # [4, ash]
        c_i = c_i.reshape(4, ACH, 128).transpose(0, 2, 1)        # [4,128,ACH]
        a_i = inv_area[i * ash:(i + 1) * ash].reshape(ACH, 128).T  # [128,ACH]
        in_maps.append({
            "fm": fm_i,
            "fcw": fcwT,
            "tri": tri,
            "trib": tri.astype(fdt),
            "cidx": np.ascontiguousarray(c_i),
            "iar": np.ascontiguousarray(a_i),
        })
    return in_maps, ACH, N, Npad, K


def kernel(**inputs):
    global LAST_RESULTS
    feature_map = np.asarray(inputs["feature_map"], dtype=np.float32)
    scale = np.asarray(inputs["scale"], dtype=np.float32)
    anchors = np.asarray(inputs["anchors"], dtype=np.float32)
    fc_w = np.asarray(inputs["fc_w"], dtype=np.float32)
    fc_b = np.asarray(inputs["fc_b"], dtype=np.float32)
    anchor_num = int(np.asarray(inputs["anchor_num"]))

    C, H, W = feature_map.shape
    K = fc_w.shape[0]

    import time
    t0 = time.time()
    bf16 = bool(int(os.environ.get("NMS_BF16", "1")))
    in_maps, ACH, N, Npad, K = _prepare(feature_map, scale, anchors, fc_w,
                                        anchor_num, bf16)
    print(f"[kernel] host prep {time.time() - t0:.1f}s", flush=True)
    gather_batch = int(os.environ.get("NMS_GATHER_BATCH", "1"))
    while ACH % gather_batch:
        gather_batch //= 2
    t0 = time.time()
    nc = _get_nc(C, H, W, K, ACH, gather_batch, bf16)
    print(f"[kernel] bass build+schedule {time.time() - t0:.1f}s", flush=True)

    from concourse.bass_utils import run_bass_kernel_spmd
    trace = bool(int(os.environ.get("NMS_TRACE", "0")))
    t0 = time.time()
    res = run_bass_kernel_spmd(nc, in_maps, core_ids=list(range(NCORES)),
                               trace=trace)
    print(f"[kernel] compile+run {time.time() - t0:.1f}s", flush=True)
    LAST_RESULTS = res
    pred = np.concatenate([res.results[i]["pred"] for i in range(NCORES)],
                          axis=0)[:N]
    return (pred + fc_b[None, :].astype(np.float32)).astype(np.float32)


# revision 15
# speedup vs baseline: 1.4256x; 1.0011x over previous
"""Bass/Trainium2 kernel for nn_FC_Classifier (box-pooled FC classifier).

Math: pred[n,k] = (1/area_n) * sum_{(h,w) in box_n} (fc_w @ feature_map)[k,h,w] + fc_b[k]

Strategy (8 cores, one chip):
  * Contract channels FIRST (2048 -> 150) with a matmul: G[w,h,k] — sharded
    over image rows h (24 rows/core).  This is the only phase that touches the
    302 MB feature map, so it is HBM-bound and fully parallel.
  * W-cumsum via triangular matmul (PE): Q[x,h,k], still h-sharded.
  * AllToAll: trade h-shards for x-shards (compact 2.9 MB buffers).
  * H-cumsum via triangular matmul: T[y, x_shard, k] = integral image of G.
  * AllGather the x-shards into the full integral image T[x,y,k] (shared buf).
  * 4-corner indirect-DMA gathers at host-precomputed box corners, combine on
    DVE, scale by 1/area; bias is added on the host.

Self-contained: only numpy + the concourse (Bass) runtime are imported.
"""

import os
import numpy as np

DS = 8.0
NCORES = 8

LAST_RESULTS = None  # BassKernelResults of the most recent run (for test.py)

_NC_CACHE = {}


def _chunks(total, size):
    return [(o, min(size, total - o)) for o in range(0, total, size)]


def _box_indices_np(anchors, scale, H, W):
    # exact replica of reference._box_indices in numpy f32
    a = anchors.astype(np.float32) / np.float32(DS)
    x0 = (a[:, 0] * scale[1]).astype(np.int32)
    x1 = (a[:, 1] * scale[1]).astype(np.int32)
    y0 = (a[:, 2] * scale[0]).astype(np.int32)
    y1 = (a[:, 3] * scale[0]).astype(np.int32)
    eqy = y0 == y1
    y0, y1 = (
        np.where(eqy & (y0 != 0), y0 - 1, y0),
        np.where(eqy & (y0 == 0), y1 + 1, y1),
    )
    eqx = x0 == x1
    x0, x1 = (
        np.where(eqx & (x0 != 0), x0 - 1, x0),
        np.where(eqx & (x0 == 0), x1 + 1, x1),
    )
    y0, y1 = np.clip(y0, 0, H), np.clip(y1, 0, H)
    x0, x1 = np.clip(x0, 0, W), np.clip(x1, 0, W)
    return x0, x1, y0, y1


def _build_nc(C, H, W, K, ACH, gather_batch, bf16):
    """Build + compile the SPMD Bass program (identical on all 8 cores).

    ACH = anchor chunks of 128 per core.  gather_batch = corners gathered per
    indirect DMA instruction (1 = one offset column per instruction).
    bf16: feature_map/fc_w arrive as bf16 (1-pass matmuls + FWL, half the
    HBM traffic); the integral-image math stays f32.
    """
    from concourse import bacc, mybir, tile
    import concourse.bass as bass

    f32 = mybir.dt.float32
    fmdt = mybir.dt.bfloat16 if bf16 else f32
    i32 = mybir.dt.int32

    HSH = H // NCORES              # h rows per core
    YD = H + 1                     # y index range of integral image
    XP = -(-(W + 1) // NCORES) * NCORES  # x range padded to multiple of 8
    XSH = XP // NCORES             # x cols per core after AllToAll
    CCH = C // 128                 # channel chunks
    assert C % 128 == 0 and H % NCORES == 0

    wch = _chunks(W, 128)          # w partition chunks for G
    xch = _chunks(XP, 128)         # x partition chunks for Q
    ych = _chunks(YD, 128)         # y partition chunks for T
    # h-groups for the H-cumsum contraction: whole source blocks (HSH rows
    # each) packed into <=128 partitions
    bpg = max(1, 128 // HSH)       # blocks per group
    hgrp = _chunks(NCORES, bpg)    # (block_off, nblocks)

    NF2 = HSH * K                  # free size of G/Q tiles
    NF3 = XSH * K                  # free size of R/T tiles

    nc = bacc.Bacc("TRN2", target_bir_lowering=False, debug=False,
                   num_devices=NCORES)
    fm = nc.dram_tensor("fm", [C, HSH, W], fmdt, kind="ExternalInput").ap()
    fcw = nc.dram_tensor("fcw", [C, K], fmdt, kind="ExternalInput").ap()
    tri = nc.dram_tensor("tri", [W, XP], f32, kind="ExternalInput").ap()
    trib = nc.dram_tensor("trib", [W, XP], fmdt, kind="ExternalInput").ap()
    cidx = nc.dram_tensor("cidx", [4, 128, ACH], i32, kind="ExternalInput").ap()
    iar = nc.dram_tensor("iar", [128, ACH], f32, kind="ExternalInput").ap()
    pred = nc.dram_tensor("pred", [128 * ACH, K], f32, kind="ExternalOutput").ap()

    RG = [list(range(NCORES))]

    with tile.TileContext(nc) as tc:
        with (
            tc.tile_pool(name="constp", bufs=1) as constp,
            tc.tile_pool(name="fmp", bufs=4) as fmp,
            tc.tile_pool(name="bigp", bufs=1) as bigp,
            tc.tile_pool(name="psp", bufs=6, space="PSUM") as psp,
            tc.tile_pool(name="gatp", bufs=2) as gatp,
            tc.tile_pool(name="dramp", bufs=1, space="DRAM") as dramp,
        ):
            # ---- constants -------------------------------------------------
            fcw_sb = constp.tile([128, CCH * K], fmdt, tag="fcw", name="fcw_sb")
            nc.sync.dma_start(fcw_sb[:], fcw.rearrange("(cc p) k -> p cc k", p=128))

            tri_w = []                       # [wsz, XP] per w-chunk (phase 2 lhs-K rows)
            for j, (off, sz) in enumerate(wch):
                t = constp.tile([sz, XP], fmdt, tag=f"tri_w{j}", name=f"tri_w{j}")
                nc.sync.dma_start(t[:], trib[off:off + sz, :])
                tri_w.append(t)

            tri_h = []                       # [grows, YD] per h-group (phase 4)
            for j, (boff, nb) in enumerate(hgrp):
                r0, rn = boff * HSH, nb * HSH
                t = constp.tile([rn, YD], f32, tag=f"tri_h{j}", name=f"tri_h{j}")
                nc.sync.dma_start(t[:], tri[r0:r0 + rn, 0:YD])
                tri_h.append(t)

            idx_sb = constp.tile([128, 4 * ACH], i32, tag="idx", name="idx_sb")
            nc.sync.dma_start(idx_sb[:], cidx.rearrange("c p m -> p c m"))
            iar_sb = constp.tile([128, ACH], f32, tag="iar", name="iar_sb")
            nc.sync.dma_start(iar_sb[:], iar[:, :])

            # ---- phase 1: channel contraction  G[w, (h,k)] -----------------
            G = [bigp.tile([sz, NF2], fmdt, tag=f"G{j}", name=f"G{j}")
                 for j, (off, sz) in enumerate(wch)]
            # load HB image rows per DMA so per-partition runs stay >=512B
            HB = 2 if HSH % 2 == 0 else 1
            fmv = fm.rearrange("(cc p) h w -> p cc (h w)", p=128)
            for h0 in range(0, HSH, HB):
                fmh = fmp.tile([128, CCH * HB * W], fmdt, tag="fmh", name="fmh")
                nc.sync.dma_start(fmh[:], fmv[:, :, h0 * W:(h0 + HB) * W])
                for hh in range(HB):
                    h = h0 + hh
                    for j, (woff, wsz) in enumerate(wch):
                        ps = psp.tile([wsz, K], f32, tag="ps", name="ps1")
                        for cc in range(CCH):
                            o = cc * (HB * W) + hh * W + woff
                            nc.tensor.matmul(
                                ps[:],
                                lhsT=fmh[:, o: o + wsz],
                                rhs=fcw_sb[:, cc * K:(cc + 1) * K],
                                start=(cc == 0), stop=(cc == CCH - 1),
                            )
                        nc.vector.tensor_copy(G[j][:, h * K:(h + 1) * K], ps[:])

            # ---- phase 2: W-cumsum  Q[x, (h,k)] ----------------------------
            Q = [bigp.tile([sz, NF2], f32, tag=f"Q{j}", name=f"Q{j}")
                 for j, (off, sz) in enumerate(xch)]
            for j, (xoff, xsz) in enumerate(xch):
                for n0, nsz in _chunks(NF2, 512):
                    ps = psp.tile([xsz, nsz], f32, tag="ps", name="ps2")
                    for wj, (woff, wsz) in enumerate(wch):
                        nc.tensor.matmul(
                            ps[:],
                            lhsT=tri_w[wj][:, xoff:xoff + xsz],
                            rhs=G[wj][:, n0:n0 + nsz],
                            start=(wj == 0), stop=(wj == len(wch) - 1),
                        )
                    nc.vector.tensor_copy(Q[j][:, n0:n0 + nsz], ps[:])

            # ---- AllToAll: h-shards -> x-shards ----------------------------
            a2a_in = dramp.tile([XP, NF2], f32, tag="a2a_in", name="a2a_in")
            a2a_out = dramp.tile([NCORES, XSH, HSH, K], f32, tag="a2a_out",
                                 name="a2a_out")
            for j, (xoff, xsz) in enumerate(xch):
                nc.sync.dma_start(a2a_in[xoff:xoff + xsz, :], Q[j][:])
            nc.gpsimd.collective_compute(
                "AllToAll", mybir.AluOpType.bypass, replica_groups=RG,
                ins=[a2a_in.opt()], outs=[a2a_out.opt()],
            )

            # ---- phase 3: H-cumsum  T[y, (x,k)] ----------------------------
            # R/T tiles reuse the G/Q slots (G and Q are dead by now)
            aov = a2a_out.rearrange("i x h k -> i h x k")
            R = []
            for j, (boff, nb) in enumerate(hgrp):
                t = bigp.tile([nb * HSH, NF3], f32, tag=f"G{j % len(wch)}",
                              name=f"R{j}")
                for b in range(nb):
                    nc.sync.dma_start(t[b * HSH:(b + 1) * HSH, :],
                                      aov[boff + b])
                R.append(t)
            T = [bigp.tile([sz, NF3], f32, tag=f"Q{j % len(xch)}", name=f"T{j}")
                 for j, (off, sz) in enumerate(ych)]
            for j, (yoff, ysz) in enumerate(ych):
                for n0, nsz in _chunks(NF3, 512):
                    ps = psp.tile([ysz, nsz], f32, tag="ps", name="ps3")
                    for gj in range(len(hgrp)):
                        nc.tensor.matmul(
                            ps[:],
                            lhsT=tri_h[gj][:, yoff:yoff + ysz],
                            rhs=R[gj][:, n0:n0 + nsz],
                            start=(gj == 0), stop=(gj == len(hgrp) - 1),
                        )
                    nc.vector.tensor_copy(T[j][:, n0:n0 + nsz], ps[:])

            # ---- AllGather full integral image T[x, y, k] ------------------
            ag_in = dramp.tile([XSH, YD, K], f32, tag="ag_in", name="ag_in")
            ag_out = dramp.tile([XP * YD, K], f32, tag="ag_out", name="ag_out",
                                addr_space="Shared")
            agv = ag_in.rearrange("x y k -> y x k")
            for j, (yoff, ysz) in enumerate(ych):
                nc.sync.dma_start(agv[yoff:yoff + ysz], T[j][:])
            nc.gpsimd.collective_compute(
                "AllGather", mybir.AluOpType.bypass, replica_groups=RG,
                ins=[ag_in.opt()], outs=[ag_out.opt()],
            )

            # ---- phase 4: corner gathers + combine -------------------------
            GB = gather_batch
            assert ACH % GB == 0
            pv = pred.rearrange("(m p) k -> p m k", p=128)
            for m0 in range(0, ACH, GB):
                g = []
                for c in range(4):
                    gt = gatp.tile([128, GB * K], f32, tag=f"g{c}", name=f"g{c}")
                    nc.gpsimd.indirect_dma_start(
                        out=gt[:],
                        out_offset=None,
                        in_=ag_out[:],
                        in_offset=bass.IndirectOffsetOnAxis(
                            ap=idx_sb[:, c * ACH + m0: c * ACH + m0 + GB],
                            axis=0,
                        ),
                    )
                    g.append(gt)
                # sums = g0 - g1 - g2 + g3, scaled by 1/area (in-place combine)
                nc.vector.tensor_sub(g[0][:], g[0][:], g[1][:])
                nc.vector.tensor_sub(g[2][:], g[2][:], g[3][:])
                nc.vector.tensor_sub(g[0][:], g[0][:], g[2][:])
                for b in range(GB):
                    nc.vector.tensor_scalar_mul(
                        g[1][:, b * K:(b + 1) * K], g[0][:, b * K:(b + 1) * K],
                        iar_sb[:, m0 + b: m0 + b + 1],
                    )
                # pred rows: anchor a = m*128 + p
                nc.sync.dma_start(pv[:, m0:m0 + GB, :], g[1][:])

    nc.compile()
    return nc


def _get_nc(C, H, W, K, ACH, gather_batch, bf16):
    key = (C, H, W, K, ACH, gather_batch, bf16)
    if key not in _NC_CACHE:
        _NC_CACHE[key] = _build_nc(C, H, W, K, ACH, gather_batch, bf16)
    return _NC_CACHE[key]


def _prepare(feature_map, scale, anchors, fc_w, anchor_num, bf16):
    """Host-side prep: shard fm, build tri matrix, corner indices, areas."""
    C, H, W = feature_map.shape
    K = fc_w.shape[0]
    HSH = H // NCORES
    YD = H + 1
    XP = -(-(W + 1) // NCORES) * NCORES

    N = int(anchor_num)
    anchors = np.asarray(anchors, dtype=np.float32)[:N]
    per = 128 * NCORES
    Npad = max(per, -(-N // per) * per)
    if Npad != N:
        pad = np.zeros((Npad - N, 4), dtype=np.float32)
        anchors = np.concatenate([anchors, pad], axis=0)
    ACH = Npad // per

    x0, x1, y0, y1 = _box_indices_np(anchors, np.asarray(scale, np.float32), H, W)
    area = np.maximum((y1 - y0) * (x1 - x0), 1).astype(np.float32)
    inv_area = (np.float32(1.0) / area).astype(np.float32)
    # integral image is stored [x, y, k]; row id = x*YD + y
    corners = np.stack([
        x1 * YD + y1,
        x1 * YD + y0,
        x0 * YD + y1,
        x0 * YD + y0,
    ]).astype(np.int32)                      # [4, Npad]

    if bf16:
        import ml_dtypes
        fdt = ml_dtypes.bfloat16
    else:
        fdt = np.float32
    fcwT = np.ascontiguousarray(fc_w.T.astype(fdt))              # [C, K]
    tri = np.zeros((W, XP), dtype=np.float32)
    for x in range(1, W + 1):
        tri[0:x, x] = 1.0

    ash = Npad // NCORES
    in_maps = []
    for i in range(NCORES):
        fm_i = np.ascontiguousarray(feature_map[:, i * HSH:(i + 1) * HSH, :].astype(fdt))
        c_i = corners[:, i * ash:(i + 1) * ash]                  # [4, ash]
        c_i = c_i.reshape(4, ACH, 128).transpose(0, 2, 1)        # [4,128,ACH]
        a_i = inv_area[i * ash:(i + 1) * ash].reshape(ACH, 128).T  # [128,ACH]
        in_maps.append({
            "fm": fm_i,
            "fcw": fcwT,
            "tri": tri,
            "trib": tri.astype(fdt),
            "cidx": np.ascontiguousarray(c_i),
            "iar": np.ascontiguousarray(a_i),
        })
    return in_maps, ACH, N, Npad, K


def kernel(**inputs):
    global LAST_RESULTS
    feature_map = np.asarray(inputs["feature_map"], dtype=np.float32)
    scale = np.asarray(inputs["scale"], dtype=np.float32)
    anchors = np.asarray(inputs["anchors"], dtype=np.float32)
    fc_w = np.asarray(inputs["fc_w"], dtype=np.float32)
    fc_b = np.asarray(inputs["fc_b"], dtype=np.float32)
    anchor_num = int(np.asarray(inputs["anchor_num"]))

    C, H, W = feature_map.shape
    K = fc_w.shape[0]

    import time
    t0 = time.time()
    bf16 = bool(int(os.environ.get("NMS_BF16", "1")))
    in_maps, ACH, N, Npad, K = _prepare(feature_map, scale, anchors, fc_w,
                                        anchor_num, bf16)
    print(f"[kernel] host prep {time.time() - t0:.1f}s", flush=True)
    gather_batch = int(os.environ.get("NMS_GATHER_BATCH", "1"))
    while ACH % gather_batch:
        gather_batch //= 2
    t0 = time.time()
    nc = _get_nc(C, H, W, K, ACH, gather_batch, bf16)
    print(f"[kernel] bass build+schedule {time.time() - t0:.1f}s", flush=True)

    from concourse.bass_utils import run_bass_kernel_spmd
    trace = bool(int(os.environ.get("NMS_TRACE", "0")))
    t0 = time.time()
    res = run_bass_kernel_spmd(nc, in_maps, core_ids=list(range(NCORES)),
                               trace=trace)
    print(f"[kernel] compile+run {time.time() - t0:.1f}s", flush=True)
    LAST_RESULTS = res
    pred = np.concatenate([res.results[i]["pred"] for i in range(NCORES)],
                          axis=0)[:N]
    return (pred + fc_b[None, :].astype(np.float32)).astype(np.float32)


# revision 16
# speedup vs baseline: 1.4654x; 1.0280x over previous
"""Bass/Trainium2 kernel for nn_FC_Classifier (box-pooled FC classifier).

Math: pred[n,k] = (1/area_n) * sum_{(h,w) in box_n} (fc_w @ feature_map)[k,h,w] + fc_b[k]

Strategy (8 cores, one chip):
  * Contract channels FIRST (2048 -> 150) with a matmul: G[w,h,k] — sharded
    over image rows h (24 rows/core).  This is the only phase that touches the
    302 MB feature map, so it is HBM-bound and fully parallel.
  * W-cumsum via triangular matmul (PE): Q[x,h,k], still h-sharded.
  * AllToAll: trade h-shards for x-shards (compact 2.9 MB buffers).
  * H-cumsum via triangular matmul: T[y, x_shard, k] = integral image of G.
  * AllGather the x-shards into the full integral image T[x,y,k] (shared buf).
  * 4-corner indirect-DMA gathers at host-precomputed box corners, combine on
    DVE, scale by 1/area; bias is added on the host.

Self-contained: only numpy + the concourse (Bass) runtime are imported.
"""

import os
import numpy as np

DS = 8.0
NCORES = 8

LAST_RESULTS = None  # BassKernelResults of the most recent run (for test.py)

_NC_CACHE = {}


def _chunks(total, size):
    return [(o, min(size, total - o)) for o in range(0, total, size)]


def _box_indices_np(anchors, scale, H, W):
    # exact replica of reference._box_indices in numpy f32
    a = anchors.astype(np.float32) / np.float32(DS)
    x0 = (a[:, 0] * scale[1]).astype(np.int32)
    x1 = (a[:, 1] * scale[1]).astype(np.int32)
    y0 = (a[:, 2] * scale[0]).astype(np.int32)
    y1 = (a[:, 3] * scale[0]).astype(np.int32)
    eqy = y0 == y1
    y0, y1 = (
        np.where(eqy & (y0 != 0), y0 - 1, y0),
        np.where(eqy & (y0 == 0), y1 + 1, y1),
    )
    eqx = x0 == x1
    x0, x1 = (
        np.where(eqx & (x0 != 0), x0 - 1, x0),
        np.where(eqx & (x0 == 0), x1 + 1, x1),
    )
    y0, y1 = np.clip(y0, 0, H), np.clip(y1, 0, H)
    x0, x1 = np.clip(x0, 0, W), np.clip(x1, 0, W)
    return x0, x1, y0, y1


def _build_nc(C, H, W, K, ACH, gather_batch, bf16):
    """Build + compile the SPMD Bass program (identical on all 8 cores).

    ACH = anchor chunks of 128 per core.  gather_batch = corners gathered per
    indirect DMA instruction (1 = one offset column per instruction).
    bf16: feature_map/fc_w arrive as bf16 (1-pass matmuls + FWL, half the
    HBM traffic); the integral-image math stays f32.
    """
    from concourse import bacc, mybir, tile
    import concourse.bass as bass

    f32 = mybir.dt.float32
    fmdt = mybir.dt.bfloat16 if bf16 else f32
    i32 = mybir.dt.int32

    HSH = H // NCORES              # h rows per core
    YD = H + 1                     # y index range of integral image
    XP = -(-(W + 1) // NCORES) * NCORES  # x range padded to multiple of 8
    XSH = XP // NCORES             # x cols per core after AllToAll
    CCH = C // 128                 # channel chunks
    assert C % 128 == 0 and H % NCORES == 0

    wch = _chunks(W, 128)          # w partition chunks for G
    xch = _chunks(XP, 128)         # x partition chunks for Q
    ych = _chunks(YD, 128)         # y partition chunks for T
    # h-groups for the H-cumsum contraction: whole source blocks (HSH rows
    # each) packed into <=128 partitions
    bpg = max(1, 128 // HSH)       # blocks per group
    hgrp = _chunks(NCORES, bpg)    # (block_off, nblocks)

    NF2 = HSH * K                  # free size of G/Q tiles
    NF3 = XSH * K                  # free size of R/T tiles

    nc = bacc.Bacc("TRN2", target_bir_lowering=False, debug=False,
                   num_devices=NCORES)
    fm = nc.dram_tensor("fm", [C, HSH, W], fmdt, kind="ExternalInput").ap()
    fcw = nc.dram_tensor("fcw", [C, K], fmdt, kind="ExternalInput").ap()
    tri = nc.dram_tensor("tri", [W, XP], f32, kind="ExternalInput").ap()
    trib = nc.dram_tensor("trib", [W, XP], fmdt, kind="ExternalInput").ap()
    cidx = nc.dram_tensor("cidx", [4, 128, ACH], i32, kind="ExternalInput").ap()
    iar = nc.dram_tensor("iar", [128, ACH], f32, kind="ExternalInput").ap()
    pred = nc.dram_tensor("pred", [128 * ACH, K], f32, kind="ExternalOutput").ap()

    RG = [list(range(NCORES))]

    with tile.TileContext(nc) as tc:
        with (
            tc.tile_pool(name="constp", bufs=1) as constp,
            tc.tile_pool(name="fmp", bufs=4) as fmp,
            tc.tile_pool(name="bigp", bufs=1) as bigp,
            tc.tile_pool(name="psp", bufs=6, space="PSUM") as psp,
            tc.tile_pool(name="gatp", bufs=3) as gatp,
            tc.tile_pool(name="dramp", bufs=1, space="DRAM") as dramp,
        ):
            # ---- constants -------------------------------------------------
            fcw_sb = constp.tile([128, CCH * K], fmdt, tag="fcw", name="fcw_sb")
            nc.sync.dma_start(fcw_sb[:], fcw.rearrange("(cc p) k -> p cc k", p=128))

            tri_w = []                       # [wsz, XP] per w-chunk (phase 2 lhs-K rows)
            for j, (off, sz) in enumerate(wch):
                t = constp.tile([sz, XP], fmdt, tag=f"tri_w{j}", name=f"tri_w{j}")
                nc.sync.dma_start(t[:], trib[off:off + sz, :])
                tri_w.append(t)

            tri_h = []                       # [grows, YD] per h-group (phase 4)
            for j, (boff, nb) in enumerate(hgrp):
                r0, rn = boff * HSH, nb * HSH
                t = constp.tile([rn, YD], f32, tag=f"tri_h{j}", name=f"tri_h{j}")
                nc.sync.dma_start(t[:], tri[r0:r0 + rn, 0:YD])
                tri_h.append(t)

            idx_sb = constp.tile([128, 4 * ACH], i32, tag="idx", name="idx_sb")
            nc.sync.dma_start(idx_sb[:], cidx.rearrange("c p m -> p c m"))
            iar_sb = constp.tile([128, ACH], f32, tag="iar", name="iar_sb")
            nc.sync.dma_start(iar_sb[:], iar[:, :])

            # ---- phase 1: channel contraction  G[w, (h,k)] -----------------
            G = [bigp.tile([sz, NF2], fmdt, tag=f"G{j}", name=f"G{j}")
                 for j, (off, sz) in enumerate(wch)]
            # load HB image rows per DMA so per-partition runs stay >=512B
            HB = 2 if HSH % 2 == 0 else 1
            fmv = fm.rearrange("(cc p) h w -> p cc (h w)", p=128)
            for h0 in range(0, HSH, HB):
                fmh = fmp.tile([128, CCH * HB * W], fmdt, tag="fmh", name="fmh")
                nc.sync.dma_start(fmh[:], fmv[:, :, h0 * W:(h0 + HB) * W])
                for hh in range(HB):
                    h = h0 + hh
                    for j, (woff, wsz) in enumerate(wch):
                        ps = psp.tile([wsz, K], f32, tag="ps", name="ps1")
                        for cc in range(CCH):
                            o = cc * (HB * W) + hh * W + woff
                            nc.tensor.matmul(
                                ps[:],
                                lhsT=fmh[:, o: o + wsz],
                                rhs=fcw_sb[:, cc * K:(cc + 1) * K],
                                start=(cc == 0), stop=(cc == CCH - 1),
                            )
                        nc.vector.tensor_copy(G[j][:, h * K:(h + 1) * K], ps[:])

            # ---- phase 2: W-cumsum  Q[x, (h,k)] ----------------------------
            Q = [bigp.tile([sz, NF2], f32, tag=f"Q{j}", name=f"Q{j}")
                 for j, (off, sz) in enumerate(xch)]
            for j, (xoff, xsz) in enumerate(xch):
                for n0, nsz in _chunks(NF2, 512):
                    ps = psp.tile([xsz, nsz], f32, tag="ps", name="ps2")
                    for wj, (woff, wsz) in enumerate(wch):
                        nc.tensor.matmul(
                            ps[:],
                            lhsT=tri_w[wj][:, xoff:xoff + xsz],
                            rhs=G[wj][:, n0:n0 + nsz],
                            start=(wj == 0), stop=(wj == len(wch) - 1),
                        )
                    nc.vector.tensor_copy(Q[j][:, n0:n0 + nsz], ps[:])

            # ---- AllToAll: h-shards -> x-shards ----------------------------
            a2a_in = dramp.tile([XP, NF2], f32, tag="a2a_in", name="a2a_in")
            a2a_out = dramp.tile([NCORES, XSH, HSH, K], f32, tag="a2a_out",
                                 name="a2a_out")
            for j, (xoff, xsz) in enumerate(xch):
                nc.sync.dma_start(a2a_in[xoff:xoff + xsz, :], Q[j][:])
            nc.gpsimd.collective_compute(
                "AllToAll", mybir.AluOpType.bypass, replica_groups=RG,
                ins=[a2a_in.opt()], outs=[a2a_out.opt()],
            )

            # ---- phase 3: H-cumsum  T[y, (x,k)] ----------------------------
            # R/T tiles reuse the G/Q slots (G and Q are dead by now)
            aov = a2a_out.rearrange("i x h k -> i h x k")
            R = []
            for j, (boff, nb) in enumerate(hgrp):
                t = bigp.tile([nb * HSH, NF3], f32, tag=f"G{j % len(wch)}",
                              name=f"R{j}")
                for b in range(nb):
                    nc.sync.dma_start(t[b * HSH:(b + 1) * HSH, :],
                                      aov[boff + b])
                R.append(t)
            T = [bigp.tile([sz, NF3], f32, tag=f"Q{j % len(xch)}", name=f"T{j}")
                 for j, (off, sz) in enumerate(ych)]
            for j, (yoff, ysz) in enumerate(ych):
                for n0, nsz in _chunks(NF3, 512):
                    ps = psp.tile([ysz, nsz], f32, tag="ps", name="ps3")
                    for gj in range(len(hgrp)):
                        nc.tensor.matmul(
                            ps[:],
                            lhsT=tri_h[gj][:, yoff:yoff + ysz],
                            rhs=R[gj][:, n0:n0 + nsz],
                            start=(gj == 0), stop=(gj == len(hgrp) - 1),
                        )
                    nc.vector.tensor_copy(T[j][:, n0:n0 + nsz], ps[:])

            # ---- AllGather full integral image, block-major [j][y][xl][k] --
            # y-major per-rank blocks make the T->DRAM write contiguous per
            # partition (15 KB runs instead of 600 B strided descriptors) and
            # each n-chunk is written as soon as the H-cumsum produces it.
            ag_in = dramp.tile([YD, XSH, K], f32, tag="ag_in", name="ag_in")
            ag_out = dramp.tile([XP * YD, K], f32, tag="ag_out", name="ag_out",
                                addr_space="Shared")
            agv = ag_in.rearrange("y x k -> y (x k)")
            for j, (yoff, ysz) in enumerate(ych):
                for n0, nsz in _chunks(NF3, 512):
                    nc.sync.dma_start(agv[yoff:yoff + ysz, n0:n0 + nsz],
                                      T[j][:, n0:n0 + nsz])
            nc.gpsimd.collective_compute(
                "AllGather", mybir.AluOpType.bypass, replica_groups=RG,
                ins=[ag_in.opt()], outs=[ag_out.opt()],
            )

            # ---- phase 4: corner gathers + combine -------------------------
            GB = gather_batch
            assert ACH % GB == 0
            pv = pred.rearrange("(m p) k -> p m k", p=128)
            for m0 in range(0, ACH, GB):
                g = []
                for c in range(4):
                    gt = gatp.tile([128, GB * K], f32, tag=f"g{c}", name=f"g{c}")
                    nc.gpsimd.indirect_dma_start(
                        out=gt[:],
                        out_offset=None,
                        in_=ag_out[:],
                        in_offset=bass.IndirectOffsetOnAxis(
                            ap=idx_sb[:, c * ACH + m0: c * ACH + m0 + GB],
                            axis=0,
                        ),
                    )
                    g.append(gt)
                # sums = g0 - g1 - g2 + g3, scaled by 1/area (in-place combine)
                nc.vector.tensor_sub(g[0][:], g[0][:], g[1][:])
                nc.vector.tensor_sub(g[2][:], g[2][:], g[3][:])
                nc.vector.tensor_sub(g[0][:], g[0][:], g[2][:])
                for b in range(GB):
                    nc.vector.tensor_scalar_mul(
                        g[1][:, b * K:(b + 1) * K], g[0][:, b * K:(b + 1) * K],
                        iar_sb[:, m0 + b: m0 + b + 1],
                    )
                # pred rows: anchor a = m*128 + p
                nc.sync.dma_start(pv[:, m0:m0 + GB, :], g[1][:])

    nc.compile()
    return nc


def _get_nc(C, H, W, K, ACH, gather_batch, bf16):
    key = (C, H, W, K, ACH, gather_batch, bf16)
    if key not in _NC_CACHE:
        _NC_CACHE[key] = _build_nc(C, H, W, K, ACH, gather_batch, bf16)
    return _NC_CACHE[key]


def _prepare(feature_map, scale, anchors, fc_w, anchor_num, bf16):
    """Host-side prep: shard fm, build tri matrix, corner indices, areas."""
    C, H, W = feature_map.shape
    K = fc_w.shape[0]
    HSH = H // NCORES
    YD = H + 1
    XP = -(-(W + 1) // NCORES) * NCORES

    N = int(anchor_num)
    anchors = np.asarray(anchors, dtype=np.float32)[:N]
    per = 128 * NCORES
    Npad = max(per, -(-N // per) * per)
    if Npad != N:
        pad = np.zeros((Npad - N, 4), dtype=np.float32)
        anchors = np.concatenate([anchors, pad], axis=0)
    ACH = Npad // per

    x0, x1, y0, y1 = _box_indices_np(anchors, np.asarray(scale, np.float32), H, W)
    area = np.maximum((y1 - y0) * (x1 - x0), 1).astype(np.float32)
    inv_area = (np.float32(1.0) / area).astype(np.float32)
    # integral image is stored as 8 rank-blocks of [YD, XSH, K]:
    # row id = (x//XSH)*(YD*XSH) + y*XSH + x%XSH
    XSH = XP // NCORES

    def rid(x, y):
        return (x // XSH) * (YD * XSH) + y * XSH + (x % XSH)

    corners = np.stack([
        rid(x1, y1),
        rid(x1, y0),
        rid(x0, y1),
        rid(x0, y0),
    ]).astype(np.int32)                      # [4, Npad]

    if bf16:
        import ml_dtypes
        fdt = ml_dtypes.bfloat16
    else:
        fdt = np.float32
    fcwT = np.ascontiguousarray(fc_w.T.astype(fdt))              # [C, K]
    tri = np.zeros((W, XP), dtype=np.float32)
    for x in range(1, W + 1):
        tri[0:x, x] = 1.0

    ash = Npad // NCORES
    in_maps = []
    for i in range(NCORES):
        fm_i = np.ascontiguousarray(feature_map[:, i * HSH:(i + 1) * HSH, :].astype(fdt))
        c_i = corners[:, i * ash:(i + 1) * ash]                  # [4, ash]
        c_i = c_i.reshape(4, ACH, 128).transpose(0, 2, 1)        # [4,128,ACH]
        a_i = inv_area[i * ash:(i + 1) * ash].reshape(ACH, 128).T  # [128,ACH]
        in_maps.append({
            "fm": fm_i,
            "fcw": fcwT,
            "tri": tri,
            "trib": tri.astype(fdt),
            "cidx": np.ascontiguousarray(c_i),
            "iar": np.ascontiguousarray(a_i),
        })
    return in_maps, ACH, N, Npad, K


def kernel(**inputs):
    global LAST_RESULTS
    feature_map = np.asarray(inputs["feature_map"], dtype=np.float32)
    scale = np.asarray(inputs["scale"], dtype=np.float32)
    anchors = np.asarray(inputs["anchors"], dtype=np.float32)
    fc_w = np.asarray(inputs["fc_w"], dtype=np.float32)
    fc_b = np.asarray(inputs["fc_b"], dtype=np.float32)
    anchor_num = int(np.asarray(inputs["anchor_num"]))

    C, H, W = feature_map.shape
    K = fc_w.shape[0]

    import time
    t0 = time.time()
    bf16 = bool(int(os.environ.get("NMS_BF16", "1")))
    in_maps, ACH, N, Npad, K = _prepare(feature_map, scale, anchors, fc_w,
                                        anchor_num, bf16)
    print(f"[kernel] host prep {time.time() - t0:.1f}s", flush=True)
    gather_batch = int(os.environ.get("NMS_GATHER_BATCH", "1"))
    while ACH % gather_batch:
        gather_batch //= 2
    t0 = time.time()
    nc = _get_nc(C, H, W, K, ACH, gather_batch, bf16)
    print(f"[kernel] bass build+schedule {time.time() - t0:.1f}s", flush=True)

    from concourse.bass_utils import run_bass_kernel_spmd
    trace = bool(int(os.environ.get("NMS_TRACE", "0")))
    t0 = time.time()
    res = run_bass_kernel_spmd(nc, in_maps, core_ids=list(range(NCORES)),
                               trace=trace)
    print(f"[kernel] compile+run {time.time() - t0:.1f}s", flush=True)
    LAST_RESULTS = res
    pred = np.concatenate([res.results[i]["pred"] for i in range(NCORES)],
                          axis=0)[:N]
    return (pred + fc_b[None, :].astype(np.float32)).astype(np.float32)
